# revision 1
# baseline (speedup 1.0000x reference)
"""Trainium2 Bass kernel: correlation (cost volume) layer.

kernel(in1, in2): full inputs [8, 256, 96, 192] f32 -> output [8, 25, 96, 192] f32.
Sharding: data-parallel over batch, one batch per NeuronCore (8 cores, SPMD).
"""
import sys
if '/opt/trn_rl_repo' not in sys.path:
    sys.path.insert(0, '/opt/trn_rl_repo')
import numpy as np

"""Correlation (cost volume) kernel for Trainium2 — verifier-legal AP edition.

out[d=(a,b), h, w] = mean_c in1[c,h,w] * in2pad[c, h+2a-4, w+2b-4],  a,b in 0..4

Design notes (walrus requires matmul stationary AND moving APs to each have
exactly ONE free dim):
- in1 parity tiles are pre-packed contiguous ([128c, 128m], m = j*8+i).
- in2 parity planes are pre-packed COLUMN-MAJOR ([128c, 100 s, 12 r]); a
  [12 x 20] halo window then spans all 12 rows of the strip-plane, so it is a
  single contiguous 240-element run -> one matmul per K-chunk, rhs AP [[1,240]].
- psum[m, n=s*12+r] band: needed entries at n = n_m + (12b + a), n_m = 12j + i.
  Sheared through DRAM scratch (per-j-block DMAs; j-block = 8 partitions with
  uniform column ranges), read back aligned, compacted 53->25 by a strided
  copy, transposed by TensorE (contiguous [[1,25]] stationary AP).
"""

import concourse.bass as bass
import concourse.mybir as mybir

f32 = mybir.dt.float32

TH, TW = 8, 16            # parity tile shape; partition m = j*8 + i
WH, WW = TH + 4, TW + 4   # window 12 x 20
NW = WH * WW              # 240
BAND = 4 * WH + 4 + 1     # 53 = span of {12b + a}
NCOLS = BAND + TH - 1     # 60 columns written per row in the shear
SHIFT0 = TH - 1           # 7
PITCH = 72                # scratch row pitch (>= SHIFT0 + NCOLS = 67)
PARITIES = ((0, 0), (0, 1), (1, 0), (1, 1))


def build_corr(nc, tc, in1_d, in2_d, out_d, scratch_d, C, H, W):
    from concourse import masks

    HP, WP = H // 2, W // 2
    NJ = WP // TW
    NSTRIP = HP // TH
    HW = H * W
    KC = C // 128
    IN2R = 2 * TH + 8             # 24 natural strip rows incl +-4 halo
    IN2C = W + 8                  # 200 natural cols incl +-4 halo
    PKC = WP + 4                  # 100 packed plane cols (+-2 parity halo)
    TILE_SCR = PITCH * 128
    NT = NJ
    NTA = (NT + 1) // 2
    inv_c = 1.0 / C

    with (
        tc.tile_pool(name="const", bufs=1) as cpool,
        tc.tile_pool(name="in1p", bufs=2) as in1_pool,
        tc.tile_pool(name="pk1", bufs=2) as pk1_pool,
        tc.tile_pool(name="pk2", bufs=2) as pk2_pool,
        tc.tile_pool(name="spool", bufs=2) as s_pool,
        tc.tile_pool(name="upool", bufs=2) as u_pool,
        tc.tile_pool(name="ypool", bufs=2) as y_pool,
        tc.tile_pool(name="opool", bufs=2) as o_pool,
        tc.tile_pool(name="psumw", bufs=4, space="PSUM") as pw_pool,
        tc.tile_pool(name="psum2", bufs=4, space="PSUM") as p2_pool,
    ):
        identity = cpool.tile([128, 128], f32)
        masks.make_identity(nc, identity[:])

        in2_bufs = []
        for ib in range(2):
            in2_buf = cpool.tile([128, KC, IN2R, IN2C], f32, tag=f"in2_{ib}")
            in2_bufs.append(in2_buf)
        for b in in2_bufs:
            nc.vector.memset(b[:, :, :, 0:4], 0.0)
            nc.vector.memset(b[:, :, :, IN2C - 4:IN2C], 0.0)

        for s in range(NSTRIP):
            # ---- load strips ----
            in1_s = in1_pool.tile([128, KC, 2 * TH, W], f32)
            for k in range(KC):
                nc.sync.dma_start(
                    in1_s[:, k],
                    bass.AP(in1_d, k * 128 * HW + s * 2 * TH * W,
                            [[HW, 128], [W, 2 * TH], [1, W]]))
            in2_s = in2_bufs[s % 2]
            r_lo = s * 2 * TH - 4
            v_lo, v_hi = max(r_lo, 0), min(r_lo + IN2R, H)
            nrow = v_hi - v_lo
            if r_lo < 0:
                nc.vector.memset(in2_s[:, :, 0:(v_lo - r_lo), 4:4 + W], 0.0)
            if r_lo + IN2R > H:
                nc.vector.memset(
                    in2_s[:, :, IN2R - (r_lo + IN2R - H):IN2R, 4:4 + W], 0.0)
            for k in range(KC):
                nc.sync.dma_start(
                    in2_s[:, k, (v_lo - r_lo):(v_lo - r_lo) + nrow, 4:4 + W],
                    bass.AP(in2_d, k * 128 * HW + v_lo * W,
                            [[HW, 128], [W, nrow], [1, W]]))

            o_sbuf = o_pool.tile([25, 2 * TH, W], f32)
            in1_ap = in1_s[:]
            in2_ap = in2_s[:]
            p_in1 = in1_ap.ap[0][0]
            p_in2 = in2_ap.ap[0][0]

            for pi, (py, px) in enumerate(PARITIES):
                # ---- pack in1 tiles contiguous: pk1[c, k, t, m], m = j*8+i ----
                pk1 = pk1_pool.tile([128, KC, NT, 128], f32, tag="pk1")
                for k in range(KC):
                    src = bass.AP(
                        in1_ap.tensor,
                        in1_ap.offset + k * 2 * TH * W + py * W + px,
                        [[p_in1, 128], [2 * TW, NT], [2, TW], [2 * W, TH]])
                    eng = nc.scalar if (k + pi) % 2 == 0 else nc.vector
                    if eng is nc.scalar:
                        nc.scalar.copy(pk1[:, k], src)
                    else:
                        nc.vector.tensor_copy(pk1[:, k], src)

                # ---- pack in2 plane column-major: pk2[c, k, s', r] ----
                pk2 = pk2_pool.tile([128, KC, PKC, WH], f32, tag="pk2")
                for k in range(KC):
                    src = bass.AP(
                        in2_ap.tensor,
                        in2_ap.offset + k * IN2R * IN2C + py * IN2C + px,
                        [[p_in2, 128], [2, PKC], [2 * IN2C, WH]])
                    eng_sc = (k + pi) % 2 == 1
                    if eng_sc:
                        nc.scalar.copy(pk2[:, k], src)
                    else:
                        nc.vector.tensor_copy(pk2[:, k], src)

                # ---- window matmuls ----
                S = s_pool.tile([128, NT, NW], f32)
                pk2_ap = pk2[:]
                p_pk2 = pk2_ap.ap[0][0]
                for t in range(NT):
                    pw = pw_pool.tile([128, NW], f32)
                    for k in range(KC):
                        rhs = bass.AP(
                            pk2_ap.tensor,
                            pk2_ap.offset + k * PKC * WH + t * TW * WH,
                            [[p_pk2, 128], [1, NW]])
                        nc.tensor.matmul(
                            pw[:], pk1[:, k, t, :], rhs,
                            start=(k == 0), stop=(k == KC - 1))
                    if t % 2 == 0:
                        nc.scalar.copy(S[:, t, :], pw[:])
                    else:
                        nc.vector.tensor_copy(S[:, t, :], pw[:])

                # ---- sheared write: one DMA per j-block of 8 partitions ----
                # m = j*8 + i; n_m = 12j + i; write cols [12j, 12j + NCOLS)
                # scratch[base + m*PITCH + SHIFT0 + (col - n_m)]
                sg_base = (s % 3) * 4 + pi
                scr_base = sg_base * NT * TILE_SCR
                s_ap = S[:]
                p_s = s_ap.ap[0][0]
                for j in range(TW):
                    sap = bass.AP(s_ap.tensor,
                                  s_ap.offset + 8 * j * p_s + WH * j,
                                  [[p_s, 8], [NW, NT], [1, NCOLS]])
                    dap = bass.AP(scratch_d,
                                  scr_base + 8 * j * PITCH + SHIFT0,
                                  [[PITCH - 1, 8], [TILE_SCR, NT], [1, NCOLS]])
                    nc.sync.dma_start(dap, sap)

                # ---- aligned read-back ----
                U = u_pool.tile([128, NT, BAND], f32)
                nc.sync.dma_start(
                    U[:],
                    bass.AP(scratch_d, scr_base + SHIFT0,
                            [[PITCH, 128], [TILE_SCR, NT], [1, BAND]]))

                # ---- compact 53 -> 25: Y[m, t, d=(a,b)] = U[m, t, 12b+a] ----
                Y = y_pool.tile([128, NT, 25], f32)
                u_ap = U[:]
                src = bass.AP(u_ap.tensor, u_ap.offset,
                              [[u_ap.ap[0][0], 128], [BAND, NT], [1, 5], [WH, 5]])
                if pi % 2 == 0:
                    nc.scalar.copy(Y[:], src)
                else:
                    nc.vector.tensor_copy(Y[:], src)

                # ---- transpose: psum2[d, m] per tile ----
                p2s = []
                for g in range(2):
                    gnt = min(NTA, NT - g * NTA)
                    p2 = p2_pool.tile([25, NTA * 128], f32, tag="p2")
                    p2s.append(p2)
                    for tt in range(gnt):
                        t = g * NTA + tt
                        nc.tensor.transpose(
                            p2[:, tt * 128:(tt + 1) * 128], Y[:, t, :],
                            identity[:])

                # ---- assemble with 1/C scaling; m=(j,i) -> h=2i+py, w=2(16t+j)+px
                o_ap = o_sbuf[:]
                p_o = o_ap.ap[0][0]
                for g in range(2):
                    gnt = min(NTA, NT - g * NTA)
                    p2_ap = p2s[g][:]
                    src = bass.AP(p2_ap.tensor, p2_ap.offset,
                                  [[p2_ap.ap[0][0], 25], [128, gnt],
                                   [TH, TW], [1, TH]])
                    dst = bass.AP(o_ap.tensor,
                                  o_ap.offset + py * W + px + g * NTA * 2 * TW,
                                  [[p_o, 25], [2 * TW, gnt], [2, TW],
                                   [2 * W, TH]])
                    if pi % 2 == 0:
                        nc.scalar.mul(dst, src, inv_c)
                    else:
                        nc.vector.tensor_scalar_mul(dst, src, inv_c)

            nc.sync.dma_start(
                bass.AP(out_d, s * 2 * TH * W, [[HW, 25], [W, 2 * TH], [1, W]]),
                o_sbuf[:])


def scratch_elems(W):
    return 12 * (W // 2 // TW) * PITCH * 128


def build_module(C=256, H=96, W=192):
    import concourse.bacc as bacc
    import concourse.tile as tile
    nc = bacc.Bacc("TRN2", target_bir_lowering=False, debug=False)
    in1_d = nc.dram_tensor("in1", [C, H, W], f32, kind="ExternalInput")
    in2_d = nc.dram_tensor("in2", [C, H, W], f32, kind="ExternalInput")
    out_d = nc.dram_tensor("out", [25, H, W], f32, kind="ExternalOutput")
    scratch_d = nc.dram_tensor("scratch", [scratch_elems(W)], f32)
    with tile.TileContext(nc) as tc:
        build_corr(nc, tc, in1_d, in2_d, out_d, scratch_d, C, H, W)
    nc.compile()
    return nc


def reference_np(in1, in2, md=4, st=2):
    import numpy as np
    in1, in2 = in1[None], in2[None]
    B, C, H, W = in1.shape
    in2p = np.pad(in2, ((0, 0), (0, 0), (md, md), (md, md)))
    outs = []
    for dy in range(0, 2 * md + 1, st):
        for dx in range(0, 2 * md + 1, st):
            outs.append((in1 * in2p[:, :, dy:dy + H, dx:dx + W]).mean(axis=1))
    return np.stack(outs, axis=1)[0]


B_FULL, C_FULL, H_FULL, W_FULL = 8, 256, 96, 192
_NC = None


def _get_nc():
    global _NC
    if _NC is None:
        _NC = build_module(C_FULL, H_FULL, W_FULL)
    return _NC


def kernel(in1, in2):
    from concourse.bass_utils import run_bass_kernel_spmd
    in1 = np.ascontiguousarray(np.asarray(in1, dtype=np.float32))
    in2 = np.ascontiguousarray(np.asarray(in2, dtype=np.float32))
    assert in1.shape == (B_FULL, C_FULL, H_FULL, W_FULL), in1.shape
    nc = _get_nc()
    in_maps = [{"in1": in1[b], "in2": in2[b]} for b in range(B_FULL)]
    res = run_bass_kernel_spmd(nc, in_maps, core_ids=list(range(B_FULL)))
    out = np.stack(
        [np.asarray(res.results[b]["out"]).reshape(25, H_FULL, W_FULL)
         for b in range(B_FULL)], axis=0)
    return out



# revision 2
# speedup vs baseline: 1.0321x; 1.0321x over previous
"""Trainium2 Bass kernel v7: correlation (cost volume) layer.

kernel(in1, in2): full inputs [8, 256, 96, 192] f32 -> output [8, 25, 96, 192] f32.
Sharding: data-parallel over batch, one batch per NeuronCore (8 cores, SPMD).

out[d=(a,b), h, w] = mean_c in1[c,h,w] * in2pad[c, h+2a-4, w+2b-4],  a,b in 0..4

v5 = v4 + dependency-stall removal (v4 had ~12us all-engine stalls at strip
boundaries from whole-tile tracking on the packed in2 plane):
- P split into TWO tiles by row parity py; the px pair stays merged inside
  each tile, so pack copies remain 4 per half-strip.  Packs for py are
  emitted right after the last parity using py finishes its matmuls, so
  they overlap the other py's compute instead of barriering the strip.
- pk1 stationary tiles are packed one parity ahead (bufs=3).
- fp16 matmuls vs resident packed plane (2-free-dim window moving APs),
  fp16 DRAM-bounce shear per parity PAIR with merged readback, fp16
  per-tile transposes into [25, 768] psum, single fused scale+assemble op
  per parity, extract skewed one pair behind compute.
"""
import sys
if '/opt/trn_rl_repo' not in sys.path:
    sys.path.insert(0, '/opt/trn_rl_repo')
import numpy as np

import concourse.bass as bass
import concourse.mybir as mybir

f32 = mybir.dt.float32
f16 = mybir.dt.float16

TH, TW = 8, 16            # parity tile shape; partition m = j*8 + i
WH, WW = TH + 4, TW + 4   # window 12 x 20
NW = WH * WW              # 240
BAND = 4 * WH + 4 + 1     # 53 = span of {12b + a}
NCOLS = BAND + TH - 1     # 60 columns written per row in the shear
SHIFT0 = TH - 1           # 7
NTP = 24                  # tiles per shear group (4 parities x 6)
RP = NTP * NCOLS + 8      # 1448 scratch row pitch
URUN = (NTP - 1) * NCOLS + BAND   # 1433 elems read back per row
PARITIES = ((0, 0), (0, 1), (1, 0), (1, 1))
SG = 3                    # rotating scratch groups (strips)


def build_corr(nc, tc, in1_d, in2_d, out_d, scratch_d, C, H, W):
    from concourse import masks

    HP, WP = H // 2, W // 2          # 48, 96
    NT = WP // TW                    # 6 tiles per strip-parity
    NSTRIP = HP // TH                # 6 strips
    HW = H * W
    KC = C // 128                    # 2
    PLS, PRS = WP + 4, HP + 4        # plane dims: s in [0,100), r in [0,52)
    PK = PLS * PRS                   # per (k,px) plane elems (5200)
    inv_c = 1.0 / C

    cnt = [0]

    _ROT = ('v', 'a', 'v', 'a', 'p')

    def rr_copy(dst, src):
        r = _ROT[cnt[0] % len(_ROT)]
        cnt[0] += 1
        if r == 'v':
            nc.vector.tensor_copy(dst, src)
        elif r == 'a':
            nc.scalar.copy(dst, src)
        else:
            nc.gpsimd.tensor_copy(dst, src)

    def rr_copy_nopool(dst, src, i):
        # PSUM sources: Pool cannot access PSUM
        if i % 2 == 0:
            nc.vector.tensor_copy(dst, src)
        else:
            nc.scalar.copy(dst, src)

    with (
        tc.tile_pool(name="const", bufs=1) as cpool,
        tc.tile_pool(name="in1p", bufs=1) as in1_pool,
        tc.tile_pool(name="in2p", bufs=1) as in2_pool,
        tc.tile_pool(name="pk1", bufs=2) as pk1_pool,
        tc.tile_pool(name="spool", bufs=2) as s_pool,
        tc.tile_pool(name="upool", bufs=3) as u_pool,
        tc.tile_pool(name="ypool", bufs=2) as y_pool,
        tc.tile_pool(name="opool", bufs=2) as o_pool,
        tc.tile_pool(name="psumw", bufs=4, space="PSUM") as pw_pool,
        tc.tile_pool(name="psum2", bufs=4, space="PSUM") as p2_pool,
    ):
        identity = cpool.tile([128, 128], f16)
        masks.make_identity(nc, identity[:])

        # resident packed in2 planes, split by row parity py;
        # inside: [c, k, px, s, r]
        P = [cpool.tile([128, KC, 2, PLS, PRS], f16, name=f"P{i}")
             for i in range(2)]
        p_aps = []
        for pt in P:
            nc.vector.memset(pt[:, :, :, :, 0:2], 0.0)
            nc.gpsimd.memset(pt[:, :, :, :, PRS - 2:PRS], 0.0)
            nc.vector.memset(pt[:, :, :, 0:2, :], 0.0)
            nc.gpsimd.memset(pt[:, :, :, PLS - 2:PLS, :], 0.0)
            p_aps.append(pt[:])

        # ---------------- in2 streaming + pack (half strips of 8 rows) -----
        def load_in2_strip(h):
            t = in2_pool.tile([128, KC, 16, W], f32, tag="in2s", name="in2s")
            nc.sync.dma_start(
                t[:],
                bass.AP(in2_d, 16 * h * W,
                        [[HW, 128], [128 * HW, KC], [1, 16 * W]]))
            return t

        def pack_in2_strip_py(t, h, py):
            # natural rows 16h..16h+15 -> parity rows 8h..8h+7; px merged
            t_ap = t[:]
            p_t = t_ap.ap[0][0]
            pp = p_aps[py]
            p_pp = pp.ap[0][0]
            for k in range(KC):
                src = bass.AP(
                    t_ap.tensor,
                    t_ap.offset + k * 16 * W + py * W,
                    [[p_t, 128], [1, 2], [2, WP], [2 * W, 8]])
                dst = bass.AP(
                    pp.tensor,
                    pp.offset + k * 2 * PK + 2 * PRS + (2 + 8 * h),
                    [[p_pp, 128], [PK, 2], [PRS, WP], [1, 8]])
                rr_copy(dst, src)

        # ---------------- in1 streaming + pack ----------------------------
        def load_in1_strip(s):
            t = in1_pool.tile([128, KC, 2 * TH, W], f32, tag="in1s",
                              name="in1s")
            nc.sync.dma_start(
                t[:],
                bass.AP(in1_d, s * 2 * TH * W,
                        [[HW, 128], [128 * HW, KC], [1, 2 * TH * W]]))
            return t

        def pack_pk1(t, par):
            py, px = PARITIES[par]
            t_ap = t[:]
            p_t = t_ap.ap[0][0]
            pk1 = pk1_pool.tile([128, KC, NT, 128], f16, tag="pk1",
                                name="pk1")
            for k in range(KC):
                src = bass.AP(
                    t_ap.tensor,
                    t_ap.offset + k * 2 * TH * W + py * W + px,
                    [[p_t, 128], [2 * TW, NT], [2, TW], [2 * W, TH]])
                rr_copy(pk1[:, k], src)
            return pk1

        # ---------------- compute one parity into S group ------------------
        def compute_parity(s, par, pk1, S, g):
            py, px = PARITIES[par]
            pp = p_aps[py]
            p_pp = pp.ap[0][0]
            for tq in range(NT // 2):
                pw = pw_pool.tile([128, 2, NW], f32, name="pw")
                for ti in range(2):
                    t = 2 * tq + ti
                    for k in range(KC):
                        rhs = bass.AP(
                            pp.tensor,
                            pp.offset + k * 2 * PK + px * PK
                            + (TW * t) * PRS + TH * s,
                            [[p_pp, 128], [PRS, WW], [1, WH]])
                        nc.tensor.matmul(
                            pw[:, ti, :], pk1[:, k, t, :], rhs,
                            start=(k == 0), stop=(k == KC - 1))
                rr_copy_nopool(
                    S[:, g * NT + 2 * tq:g * NT + 2 * tq + 2, :], pw[:],
                    tq + g)

        def shear(S, sg):
            scr_base = sg * 128 * RP
            s_ap = S[:]
            p_s = s_ap.ap[0][0]
            for j in range(TW):
                sap = bass.AP(s_ap.tensor,
                              s_ap.offset + 8 * j * p_s + WH * j,
                              [[p_s, 8], [NW, NTP], [1, NCOLS]])
                dap = bass.AP(scratch_d,
                              scr_base + 8 * j * RP + SHIFT0,
                              [[RP - 1, 8], [NCOLS, NTP], [1, NCOLS]])
                nc.sync.dma_start(dap, sap)
            U = u_pool.tile([128, URUN], f16, tag="U", name="U")
            nc.sync.dma_start(
                U[:],
                bass.AP(scratch_d, scr_base + SHIFT0, [[RP, 128], [1, URUN]]))
            return U

        # ---------------- extract: compact + transpose + assemble ----------
        def extract(s, par, U, g, o_sbuf):
            py, px = PARITIES[par]
            Y = y_pool.tile([128, NT, 25], f16, tag="Y", name="Y")
            u_ap = U[:]
            src = bass.AP(u_ap.tensor, u_ap.offset + g * NT * NCOLS,
                          [[u_ap.ap[0][0], 128], [NCOLS, NT], [1, 5],
                           [WH, 5]])
            nc.gpsimd.tensor_copy(Y[:], src)

            p2 = p2_pool.tile([25, NT * 128], f16, tag="p2", name="p2")
            for t in range(NT):
                nc.tensor.transpose(p2[:, t * 128:(t + 1) * 128],
                                    Y[:, t, :], identity[:])
            p2_ap = p2[:]
            p_p2 = p2_ap.ap[0][0]
            o_ap = o_sbuf[:]
            p_o = o_ap.ap[0][0]
            asrc = bass.AP(p2_ap.tensor, p2_ap.offset,
                           [[p_p2, 25], [128, NT], [TH, TW], [1, TH]])
            adst = bass.AP(o_ap.tensor, o_ap.offset + py * W + px,
                           [[p_o, 25], [2 * TW, NT], [2, TW], [2 * W, TH]])
            if par % 2 == 0:
                nc.scalar.mul(adst, asrc, inv_c)
            else:
                nc.vector.tensor_scalar_mul(adst, asrc, inv_c)

        # ---------------- schedule -----------------------------------------
        # in2 planes must be packed one strip ahead of compute; in1/in2
        # staging buffers are single-buffered (loads WAR-wait on the packs
        # of the previous strip, which finish early via prefetch).
        NHALF = NSTRIP
        in2_cur = [load_in2_strip(0)]
        pack_in2_strip_py(in2_cur[0], 0, 0)
        pack_in2_strip_py(in2_cur[0], 0, 1)
        in2_cur[0] = load_in2_strip(1)
        pack_in2_strip_py(in2_cur[0], 1, 0)
        pack_in2_strip_py(in2_cur[0], 1, 1)
        in1_tiles = {0: load_in1_strip(0)}

        pending = []
        o_tiles = {}
        sg_counter = 0

        def flush_one():
            ps, ppar, pU, pg = pending.pop(0)
            extract(ps, ppar, pU, pg, o_tiles[ps])
            if ppar == 3:
                ot = o_tiles.pop(ps)
                nc.sync.dma_start(
                    bass.AP(out_d, ps * 2 * TH * W,
                            [[HW, 25], [1, 2 * TH * W]]),
                    ot[:])

        pk1_next = pack_pk1(in1_tiles[0], 0)
        for s in range(NSTRIP):
            o_tiles[s] = o_pool.tile([25, 2 * TH * W], f32, tag="osb",
                                     name="osb")
            t1 = in1_tiles.pop(s)
            S = s_pool.tile([128, NTP, NW], f16, tag="S", name="S")
            h = s + 2
            if h < NHALF:
                in2_cur[0] = load_in2_strip(h)
            for par in range(4):
                pk1 = pk1_next
                if par < 3:
                    pk1_next = pack_pk1(t1, par + 1)
                    if par == 2 and s + 1 < NSTRIP:
                        in1_tiles[s + 1] = load_in1_strip(s + 1)
                elif s + 1 < NSTRIP:
                    pk1_next = pack_pk1(in1_tiles[s + 1], 0)
                compute_parity(s, par, pk1, S, par)
                if len(pending) > 3:
                    flush_one()
                # pack the in2 plane for row-parity py right after the last
                # parity using it (par 1 -> py0 free, par 3 -> py1 free)
                if h < NHALF and par in (1, 3):
                    pack_in2_strip_py(in2_cur[0], h, par // 2)
            U = shear(S, sg_counter % SG)
            sg_counter += 1
            for par in range(4):
                pending.append((s, par, U, par))
        while pending:
            flush_one()

def scratch_elems():
    return SG * 128 * RP


def build_module(C=256, H=96, W=192):
    import concourse.bacc as bacc
    import concourse.tile as tile
    nc = bacc.Bacc("TRN2", target_bir_lowering=False, debug=False)
    in1_d = nc.dram_tensor("in1", [C, H, W], f32, kind="ExternalInput")
    in2_d = nc.dram_tensor("in2", [C, H, W], f32, kind="ExternalInput")
    out_d = nc.dram_tensor("out", [25, H, W], f32, kind="ExternalOutput")
    scratch_d = nc.dram_tensor("scratch", [scratch_elems()], f16)
    with tile.TileContext(nc) as tc:
        build_corr(nc, tc, in1_d, in2_d, out_d, scratch_d, C, H, W)
    nc.compile()
    return nc


B_FULL, C_FULL, H_FULL, W_FULL = 8, 256, 96, 192
_NC = None


def _get_nc():
    global _NC
    if _NC is None:
        _NC = build_module(C_FULL, H_FULL, W_FULL)
    return _NC


def kernel(in1, in2):
    from concourse.bass_utils import run_bass_kernel_spmd
    in1 = np.ascontiguousarray(np.asarray(in1, dtype=np.float32))
    in2 = np.ascontiguousarray(np.asarray(in2, dtype=np.float32))
    assert in1.shape == (B_FULL, C_FULL, H_FULL, W_FULL), in1.shape
    nc = _get_nc()
    in_maps = [{"in1": in1[b], "in2": in2[b]} for b in range(B_FULL)]
    res = run_bass_kernel_spmd(nc, in_maps, core_ids=list(range(B_FULL)))
    out = np.stack(
        [np.asarray(res.results[b]["out"]).reshape(25, H_FULL, W_FULL)
         for b in range(B_FULL)], axis=0)
    return out
